# revision 21
# baseline (speedup 1.0000x reference)
"""KNRM-KG ranker kernel for 8 Trainium2 NeuronCores (Bass/Tile).

Strategy: pure data parallel over the batch dim (64 = 8 cores x 8 batches).
Device-side pipeline per batch:
  - transposed dma_gather of word embeddings from a per-core vocabulary-
    compacted fp16 table (rows padded to 384 cols) -> reps arrive with the
    word dim on partitions (no on-device transposes needed)
  - fp16 matmuls accumulate word + entity transforms in PSUM; bias+relu are
    fused into the PSUM->SBUF copy (tensor_scalar add+max)
  - row norms via ones-matmul partition reduction, tiny PE transposes to move
    them into doc-on-partition layout, rd = exp(-0.5*ln(nd)) on ScalarE
  - simT = d^T-chunk (stationary) @ qhat (moving) per doc chunk; the
    PSUM->SBUF copy applies rd*mask per partition and reorders (c,q)->(q,c)
  - RBF bank: shared E0 = exp(-50 s^2), per-mu Fk = exp(100 mu s - 50 mu^2),
    Gk = E0*Fk on VectorE; the sigma=0.001 kernel done directly via
    Square+Exp; free-dim reduces + ones-matmul partition reduce
  - log/mask/W_c contraction on a single partition, scores DMA'd out
"""

import numpy as np

# ---------------------------------------------------------------- constants
B, QLEN, DLEN = 64, 32, 2048
VOCAB, WORD, ENT, ATT = 50000, 300, 128, 256
NCORES = 8
BPC = B // NCORES            # batches per core
WPAD = 384                   # padded word dim (3 x 128, 768B fp16 rows)
NWCH = 3                     # word chunks of 128
NAH = 2                      # att halves of 128
NDC = DLEN // 512            # doc 512-chunks (4)
NC16 = DLEN // 128           # doc 128-chunks (16)
MUS = [-0.9, -0.7, -0.5, -0.3, -0.1, 0.1, 0.3, 0.5, 0.7, 0.9]  # sigma=0.1
NK = 12                      # 10 rbf + 1 sharp + 1 docsum row
RBF_PACK = 4                 # batches per RBF block

_CACHE = {}


# ---------------------------------------------------------------- program
def _build_program(stage=3, spk=True, hwtp=True, GSZ=512):
    import concourse.bacc as bacc
    import concourse.mybir as mybir
    import concourse.tile as tile

    fp16 = mybir.dt.float16
    f32 = mybir.dt.float32
    i16 = mybir.dt.int16
    AF = mybir.ActivationFunctionType
    ALU = mybir.AluOpType
    AX = mybir.AxisListType

    # Pin all activations to the one table set covering Exp/Ln/Square so the
    # act-table pass emits a single load instead of thrashing between sets.
    import concourse.hw_specs as hw_specs
    _orig_tables = hw_specs.get_activation_tables

    def _one_set(arch):
        t = _orig_tables(arch)
        # keep dict order (set id = index) but leave only the covering set
        return {k: (v if k == "natural_log_exp_and_others" else frozenset())
                for k, v in t.items()}

    hw_specs.get_activation_tables = _one_set
    import concourse.bacc as _bacc_mod
    _bacc_mod.get_activation_tables = _one_set
    nc = bacc.Bacc("TRN2", target_bir_lowering=False, debug=False,
                   num_devices=NCORES)

    # DRAM inputs (per-core)
    d_tab = nc.dram_tensor("tab", [17000, WPAD], fp16, kind="ExternalInput")
    d_idxd = nc.dram_tensor("idxd", [128, BPC * 128], i16, kind="ExternalInput")
    d_idxq = nc.dram_tensor("idxq", [128, 16], i16, kind="ExternalInput")
    d_entT = nc.dram_tensor("entT", [BPC, 128, DLEN], fp16, kind="ExternalInput")
    d_qentT = nc.dram_tensor("qentT", [128, BPC * QLEN], fp16, kind="ExternalInput")
    d_wt = nc.dram_tensor("wt", [128, NWCH * NAH * 128], fp16, kind="ExternalInput")
    d_we = nc.dram_tensor("we", [128, ATT], fp16, kind="ExternalInput")
    d_bias = nc.dram_tensor("bias", [128, NAH], f32, kind="ExternalInput")
    d_dmask = nc.dram_tensor("dmask", [128, BPC * NC16], f32, kind="ExternalInput")
    d_qmask = nc.dram_tensor("qmask", [1, BPC * QLEN], f32, kind="ExternalInput")
    d_wcb = nc.dram_tensor("wcb", [128, 11], fp16, kind="ExternalInput")
    d_bmask = nc.dram_tensor("bmask", [128, RBF_PACK], f32,
                             kind="ExternalInput")
    d_ident = nc.dram_tensor("ident", [16, 16], f32, kind="ExternalInput")
    d_ones16 = nc.dram_tensor("ones16", [128, 128], fp16, kind="ExternalInput")
    d_onesf = nc.dram_tensor("onesf", [1, 128], fp16, kind="ExternalInput")
    d_bc = nc.dram_tensor("bc", [1, 1], f32, kind="ExternalInput")
    d_actb = nc.dram_tensor("actb", [128, 13], f32, kind="ExternalInput")
    d_out = nc.dram_tensor("out", [1, BPC], f32, kind="ExternalOutput")

    with tile.TileContext(nc) as tc:
        with (
            tc.tile_pool(name="const", bufs=1) as cpool,
            tc.tile_pool(name="gath", bufs=2) as gpool,
            tc.tile_pool(name="ent", bufs=2) as epool,
            tc.tile_pool(name="dt", bufs=2) as dpool,
            tc.tile_pool(name="work", bufs=2) as wpool,
            tc.tile_pool(name="rbf", bufs=2) as rpool,
            tc.tile_pool(name="fin", bufs=2) as fpool,
            tc.tile_pool(name="pt", bufs=3, space="PSUM") as pt,       # transform
            tc.tile_pool(name="ps", bufs=1, space="PSUM") as ps,       # simT
            tc.tile_pool(name="pn", bufs=2, space="PSUM") as pn,       # norms
            tc.tile_pool(name="pr", bufs=1, space="PSUM") as pr,       # rd / misc
            tc.tile_pool(name="pk", bufs=1, space="PSUM") as pk,       # rbf sums
        ):
            # ---------------- load constants
            idxd = cpool.tile([128, BPC * 128], i16)
            nc.sync.dma_start(out=idxd[:], in_=d_idxd.ap())
            idxq = cpool.tile([128, 16], i16)
            nc.sync.dma_start(out=idxq[:], in_=d_idxq.ap())
            qentT = cpool.tile([128, BPC * QLEN], fp16)
            nc.sync.dma_start(out=qentT[:], in_=d_qentT.ap())
            wt = cpool.tile([128, NWCH * NAH * 128], fp16)
            nc.sync.dma_start(out=wt[:], in_=d_wt.ap())
            we = cpool.tile([128, ATT], fp16)
            nc.sync.dma_start(out=we[:], in_=d_we.ap())
            bias = cpool.tile([128, NAH], f32)
            nc.sync.dma_start(out=bias[:], in_=d_bias.ap())
            dmask = cpool.tile([128, BPC * NC16], f32)
            nc.sync.dma_start(out=dmask[:], in_=d_dmask.ap())
            qmask = cpool.tile([1, BPC * QLEN], f32)
            nc.sync.dma_start(out=qmask[:], in_=d_qmask.ap())
            wcb = cpool.tile([128, 11], fp16)
            nc.sync.dma_start(out=wcb[:], in_=d_wcb.ap())
            bmask = cpool.tile([128, RBF_PACK], f32)
            nc.sync.dma_start(out=bmask[:], in_=d_bmask.ap())
            ident = cpool.tile([16, 16], f32)
            nc.sync.dma_start(out=ident[:], in_=d_ident.ap())
            ones16 = cpool.tile([128, 128], fp16)
            nc.sync.dma_start(out=ones16[:], in_=d_ones16.ap())
            onesf = cpool.tile([1, 128], fp16)
            nc.sync.dma_start(out=onesf[:], in_=d_onesf.ap())
            bc = cpool.tile([1, 1], f32)
            nc.sync.dma_start(out=bc[:], in_=d_bc.ap())
            actb = cpool.tile([128, 13], f32)
            nc.sync.dma_start(out=actb[:], in_=d_actb.ap())

            def wt_l(wc, ah):  # W_t lhsT block [128 word, 128 att]
                o = (wc * NAH + ah) * 128
                return wt[:, o:o + 128]

            # ---------------- query side (once per core)
            qg = cpool.tile([128, NWCH, BPC * QLEN], fp16)
            nc.gpsimd.dma_gather(
                out_ap=qg[:], in_ap=d_tab.ap(), idxs_ap=idxq[:],
                num_idxs=BPC * QLEN, num_idxs_reg=BPC * QLEN,
                elem_size=WPAD, transpose=True, single_packet=spk)

            qhat = cpool.tile([128, NAH, BPC * QLEN], fp16)
            q2 = wpool.tile([128, BPC * QLEN], fp16, tag="q2")
            nq_ps = pr.tile([1, BPC * QLEN], f32, tag="small")
            for ah in range(NAH):
                qp = pt.tile([128, 512], f32, tag="tps")
                for wc in range(NWCH):
                    nc.tensor.matmul(qp[:, :BPC * QLEN], lhsT=wt_l(wc, ah),
                                     rhs=qg[:, wc, :], start=(wc == 0),
                                     stop=False)
                nc.tensor.matmul(qp[:, :BPC * QLEN],
                                 lhsT=we[:, ah * 128:(ah + 1) * 128],
                                 rhs=qentT[:], start=False, stop=True)
                # bias + relu fused
                nc.vector.tensor_scalar(
                    out=qhat[:, ah, :], in0=qp[:, :BPC * QLEN],
                    scalar1=bias[:, ah:ah + 1], scalar2=0.0,
                    op0=ALU.add, op1=ALU.max)
                nc.vector.tensor_tensor(out=q2[:], in0=qhat[:, ah, :],
                                        in1=qhat[:, ah, :], op=ALU.mult)
                nc.tensor.matmul(nq_ps[:], lhsT=ones16[:, 0:1], rhs=q2[:],
                                 start=(ah == 0), stop=(ah == 1))
            # rq = exp(-0.5*ln(nq)) * qmask  (single partition, 256 elems)
            rq = cpool.tile([1, BPC * QLEN], f32)
            nc.scalar.activation(rq[:], nq_ps[:], AF.Ln, bias=actb[0:1, 0:1])
            nc.scalar.activation(rq[:], rq[:], AF.Exp, scale=-0.5)
            nc.vector.tensor_tensor(out=rq[:], in0=rq[:], in1=qmask[:],
                                    op=ALU.mult)
            rq16 = cpool.tile([1, BPC * QLEN], fp16)
            nc.vector.tensor_copy(rq16[:], rq[:])
            rqb_ps = pr.tile([128, BPC * QLEN], f32, tag="small")
            nc.tensor.matmul(rqb_ps[:], lhsT=onesf[:], rhs=rq16[:],
                             start=True, stop=True)
            for ah in range(NAH):
                nc.vector.tensor_tensor(out=qhat[:, ah, :], in0=qhat[:, ah, :],
                                        in1=rqb_ps[:], op=ALU.mult)

            # ---------------- main loop over batches
            scores = cpool.tile([1, BPC], f32)
            for bb in range(BPC // RBF_PACK):
                S = rpool.tile([128, RBF_PACK * QLEN * 16], fp16, tag="S")
                Sv = S[:].rearrange("p (r q c) -> p r q c", r=RBF_PACK, c=16)
                for rb in range(RBF_PACK):
                    b = bb * RBF_PACK + rb
                    # gather doc reps (transposed), GSZ idxs per instruction
                    gs = []
                    ngch = DLEN // GSZ
                    for gh in range(ngch):
                        g = gpool.tile([128, NWCH, GSZ], fp16,
                                       tag=f"g{gh}", name=f"g{gh}")
                        o = b * 128 + gh * (GSZ // 16)
                        nc.gpsimd.dma_gather(
                            out_ap=g[:], in_ap=d_tab.ap(),
                            idxs_ap=idxd[:, o:o + GSZ // 16],
                            num_idxs=GSZ, num_idxs_reg=GSZ,
                            elem_size=WPAD, transpose=True, single_packet=spk)
                        gs.append(g)
                    ent = epool.tile([128, DLEN], fp16, tag="ent")
                    nc.sync.dma_start(out=ent[:], in_=d_entT.ap()[b])

                    dts = [dpool.tile([128, DLEN], fp16, tag=f"dt{ah}",
                                       name=f"dt{ah}") for ah in range(NAH)]
                    nd_sb = wpool.tile([16, DLEN], f32, tag="nd")
                    if stage < 1:
                        for ah in range(NAH):
                            nc.gpsimd.memset(dts[ah][:], 0.0)
                        nc.gpsimd.memset(nd_sb[:], 1.0)
                    for pp in range(NDC // 2 if stage >= 1 else 0):
                        dcs = (2 * pp, 2 * pp + 1)
                        sls = [slice(dc * 512, (dc + 1) * 512) for dc in dcs]
                        nd_pss = [pn.tile([16, 512], f32, tag="ndps",
                                          name="ndps") for _ in dcs]
                        for ah in range(NAH):
                            tps = [pt.tile([128, 512], f32, tag="tps",
                                           name="tps") for _ in dcs]
                            for wc in range(NWCH):
                                for j, dc in enumerate(dcs):
                                    gv = gs[dc * 512 // GSZ]
                                    go = dc * 512 % GSZ
                                    nc.tensor.matmul(
                                        tps[j][:], lhsT=wt_l(wc, ah),
                                        rhs=gv[:, wc, go:go + 512],
                                        start=(wc == 0), stop=False)
                            for j, dc in enumerate(dcs):
                                nc.tensor.matmul(
                                    tps[j][:],
                                    lhsT=we[:, ah * 128:(ah + 1) * 128],
                                    rhs=ent[:, sls[j]], start=False, stop=True)
                            for j, dc in enumerate(dcs):
                                if ah == 0:
                                    nc.vector.tensor_scalar(
                                        out=dts[ah][:, sls[j]], in0=tps[j][:],
                                        scalar1=bias[:, ah:ah + 1],
                                        scalar2=0.0, op0=ALU.add, op1=ALU.max)
                                else:
                                    nc.scalar.activation(
                                        dts[ah][:, sls[j]], tps[j][:],
                                        AF.Relu, bias=bias[:, ah:ah + 1])
                            d2 = wpool.tile([128, 1024], fp16, tag="d2")
                            pr_sl = slice(sls[0].start, sls[1].stop)
                            nc.vector.tensor_tensor(
                                out=d2[:], in0=dts[ah][:, pr_sl],
                                in1=dts[ah][:, pr_sl], op=ALU.mult)
                            for j, dc in enumerate(dcs):
                                nc.tensor.matmul(
                                    nd_pss[j][:], lhsT=ones16[:, 0:16],
                                    rhs=d2[:, j * 512:(j + 1) * 512],
                                    start=(ah == 0), stop=(ah == 1))
                        for j, dc in enumerate(dcs):
                            nc.vector.tensor_copy(nd_sb[:, sls[j]],
                                                  nd_pss[j][:])

                    # rd: transpose nd [16,2048] -> [128,16], then exp(-.5 ln)
                    rd = wpool.tile([128, 16], f32, tag="rd")
                    if stage < 1:
                        nc.gpsimd.memset(rd[:], 1.0)
                    if hwtp:
                        # HW accepts identity-column rhs: write one PSUM
                        # column per transpose, no assembly copies.
                        rd_ps = pr.tile([128, 16], f32, tag="small",
                                        name="rd_ps")
                        for c in range(NC16 if stage >= 1 else 0):
                            nc.tensor.transpose(
                                out=rd_ps[:, c:c + 1],
                                in_=nd_sb[:, c * 128:(c + 1) * 128],
                                identity=ident[:, 0:1])
                        if stage >= 1:
                            nc.scalar.activation(rd[:], rd_ps[:], AF.Ln,
                                                 bias=actb[:, 0:1])
                    else:
                        nd_t = wpool.tile([128, 16], f32, tag="nd_t")
                        for c in range(NC16 if stage >= 1 else 0):
                            tp_ps = pr.tile([128, 16], f32, tag="small",
                                            name="tp_ps")
                            nc.tensor.transpose(
                                out=tp_ps[:],
                                in_=nd_sb[:, c * 128:(c + 1) * 128],
                                identity=ident[:])
                            nc.vector.tensor_copy(nd_t[:, c:c + 1],
                                                  tp_ps[:, 0:1])
                        if stage >= 1:
                            nc.scalar.activation(rd[:], nd_t[:], AF.Ln,
                                                 bias=actb[:, 0:1])
                    if stage >= 1:
                        nc.scalar.activation(rd[:], rd[:], AF.Exp, scale=-0.5)
                        nc.vector.tensor_tensor(
                            out=rd[:], in0=rd[:],
                            in1=dmask[:, b * NC16:(b + 1) * NC16],
                            op=ALU.mult)

                    # simT: per doc-128-chunk, d^T chunk stationary, qhat moving
                    sp = ps.tile([128, 16 * QLEN], f32, tag="sps")
                    if stage < 2 and rb == 0:
                        nc.gpsimd.memset(S[:], 0.1)
                    for c in range(NC16 if stage >= 2 else 0):
                        for ah in range(NAH):
                            nc.tensor.matmul(
                                sp[:, c * QLEN:(c + 1) * QLEN],
                                lhsT=dts[ah][:, c * 128:(c + 1) * 128],
                                rhs=qhat[:, ah, b * QLEN:(b + 1) * QLEN],
                                start=(ah == 0), stop=(ah == 1))
                    # copy to S with per-partition rd*mask, reorder (c,q)->(q,c)
                    if stage >= 2:
                        spv = sp[:].rearrange("p (c q) -> p c q", c=NC16)
                        rdb = rd[:].unsqueeze(2).broadcast_to(
                            [128, NC16, QLEN])
                        sout = Sv[:, rb, :, :].transpose((0, 2, 1))
                        nc.vector.tensor_tensor(out=sout, in0=spv, in1=rdb,
                                                op=ALU.mult)

                # ---------------- RBF block on [128, RBF_PACK*32*16]
                # Doc-sum of each kernel via PE: 16 accumulating matmuls
                # (lhsT = kernel chunk [128d, 128rq] strided, rhs = ones
                # column) -> red_ps column k holds sum over all 2048 docs
                # with (r,q) on partitions.
                n = RBF_PACK * QLEN * 16
                red_ps = pk.tile([128, 16], f32, tag="red", name="red_ps")

                def docsum(k, src):
                    sv = src[:].rearrange("p (rq c) -> p rq c", c=16)
                    for c in range(16):
                        nc.tensor.matmul(red_ps[:, k:k + 1],
                                         lhsT=sv[:, :, c],
                                         rhs=ones16[:, 0:1],
                                         start=(c == 0), stop=(c == 15))

                s2 = rpool.tile([128, n], fp16, tag="s2")
                if stage >= 3:
                    nc.vector.tensor_tensor(out=s2[:], in0=S[:], in1=S[:],
                                            op=ALU.mult)
                e0 = rpool.tile([128, n], mybir.dt.bfloat16, tag="e0")
                if stage >= 3:
                    nc.scalar.activation(e0[:], s2[:], AF.Exp, scale=-50.0)
                gk = rpool.tile([128, n], fp16, tag="gk")
                for k, mu in enumerate(MUS if stage >= 3 else []):
                    fk = rpool.tile([128, n], mybir.dt.bfloat16, tag="fk")
                    nc.scalar.activation(fk[:], S[:], AF.Exp, scale=100.0 * mu,
                                         bias=actb[:, 3 + k:4 + k])
                    nc.vector.tensor_tensor(out=gk[:], in0=e0[:], in1=fk[:],
                                            op=ALU.mult)
                    docsum(k, gk)
                if stage >= 3:
                    # sharp kernel (mu=1, sigma=0.001)
                    t2 = rpool.tile([128, n], mybir.dt.bfloat16, tag="t2")
                    nc.scalar.activation(t2[:], S[:], AF.Square,
                                         scale=707.10678, bias=actb[:, 2:3])
                    e10 = rpool.tile([128, n], fp16, tag="e10")
                    nc.scalar.activation(e10[:], t2[:], AF.Exp, scale=-1.0)
                    docsum(10, e10)
                    docsum(11, S)
                else:
                    nc.gpsimd.memset(red_ps[:], 1.0)

                # final: lg = ln(red + 1e-6); w = (docsum != 0); dot W_c;
                # per-block q-sum via tiny matmul into scores_ps columns.
                eq = fpool.tile([128, 1], f32, tag="eq")
                nc.vector.tensor_scalar(out=eq[:], in0=red_ps[:, 11:12],
                                        scalar1=0.0, scalar2=None,
                                        op0=ALU.is_equal)
                w = fpool.tile([128, 1], f32, tag="w")
                nc.vector.tensor_scalar(out=w[:], in0=eq[:], scalar1=-1.0,
                                        scalar2=1.0, op0=ALU.mult, op1=ALU.add)
                lg = fpool.tile([128, 11], fp16, tag="lg")
                nc.scalar.activation(lg[:], red_ps[:, 0:11], AF.Ln,
                                     bias=actb[:, 1:2])
                p1 = fpool.tile([128, 11], fp16, tag="p1")
                nc.vector.tensor_tensor(out=p1[:], in0=lg[:], in1=wcb[:],
                                        op=ALU.mult)
                dot = fpool.tile([128, 1], f32, tag="dot")
                nc.vector.reduce_sum(out=dot[:], in_=p1[:], axis=AX.X)
                mdot = fpool.tile([128, 1], f32, tag="mdot")
                nc.vector.tensor_tensor(out=mdot[:], in0=dot[:], in1=w[:],
                                        op=ALU.mult)
                sc_ps = pr.tile([1, RBF_PACK], f32, tag="small", name="sc_ps")
                nc.tensor.matmul(sc_ps[:], lhsT=mdot[:], rhs=bmask[:],
                                 start=True, stop=True)
                nc.vector.tensor_scalar(
                    out=scores[:, bb * RBF_PACK:(bb + 1) * RBF_PACK],
                    in0=sc_ps[:], scalar1=bc[:, 0:1], scalar2=None,
                    op0=ALU.add)

            nc.sync.dma_start(out=d_out.ap(), in_=scores[:])

    try:
        nc.compile()
    finally:
        hw_specs.get_activation_tables = _orig_tables
        _bacc_mod.get_activation_tables = _orig_tables
    return nc


# ---------------------------------------------------------------- host prep
def _prep_core(core, query_tok, doc_tok, query_entity, doc_entity,
               embed_table, W_t, b_t, W_e, b_e, W_c, b_c):
    bs = slice(core * BPC, (core + 1) * BPC)
    qt = np.asarray(query_tok[bs], dtype=np.int64) + 1   # [8, 32]
    dt = np.asarray(doc_tok[bs], dtype=np.int64) + 1     # [8, 2048]

    uniq, dinv = np.unique(np.concatenate([dt.ravel(), qt.ravel()]),
                           return_inverse=True)
    nuniq = len(uniq)
    assert nuniq <= 17000
    tab = np.zeros((17000, WPAD), dtype=np.float16)
    tab[:nuniq, :WORD] = embed_table[uniq].astype(np.float16)
    didx = dinv[:BPC * DLEN].reshape(BPC, DLEN).astype(np.int16)
    qidx = dinv[BPC * DLEN:].reshape(BPC * QLEN).astype(np.int16)

    def wrap(a):  # [n] -> [128, n//16] wrapped in 16 partitions, replicated x8
        w = a.reshape(-1, 16).T  # [16, n/16]
        return np.tile(w, (8, 1)).copy()

    idxd = np.concatenate([wrap(didx[b]) for b in range(BPC)], axis=1)
    idxq = wrap(qidx)

    entT = np.ascontiguousarray(
        np.asarray(doc_entity[bs], dtype=np.float16).transpose(0, 2, 1))
    qentT = np.ascontiguousarray(
        np.asarray(query_entity[bs], dtype=np.float16).transpose(0, 2, 1)
    ).transpose(1, 0, 2).reshape(128, BPC * QLEN)
    qentT = np.ascontiguousarray(qentT)

    wtp = np.zeros((WPAD, ATT), dtype=np.float16)
    wtp[:WORD] = W_t.astype(np.float16)
    wt = np.concatenate(
        [wtp[wc * 128:(wc + 1) * 128, ah * 128:(ah + 1) * 128]
         for wc in range(NWCH) for ah in range(NAH)], axis=1)
    wt = np.ascontiguousarray(wt)
    we = np.ascontiguousarray(W_e.astype(np.float16))
    bias = np.ascontiguousarray(
        (b_t + b_e).astype(np.float32).reshape(NAH, 128).T)

    dmask = (dt != 0).astype(np.float32)  # [8, 2048]
    dmask = np.ascontiguousarray(
        dmask.reshape(BPC, NC16, 128).transpose(2, 0, 1).reshape(
            128, BPC * NC16))
    qmask = (qt != 0).astype(np.float32).reshape(1, BPC * QLEN)

    wcb = np.tile(np.asarray(W_c, dtype=np.float16).reshape(1, 11), (128, 1))
    bmask = np.zeros((128, RBF_PACK), dtype=np.float32)
    for r in range(RBF_PACK):
        bmask[r * QLEN:(r + 1) * QLEN, r] = 1.0

    return {
        "tab": tab, "idxd": idxd, "idxq": idxq, "entT": entT, "qentT": qentT,
        "wt": wt, "we": we, "bias": bias, "dmask": dmask, "qmask": qmask,
        "wcb": wcb, "bmask": bmask,
        "ident": np.eye(16, dtype=np.float32),
        "ones16": np.ones((128, 128), dtype=np.float16),
        "onesf": np.ones((1, 128), dtype=np.float16),
        "bc": np.asarray(b_c, dtype=np.float32).reshape(1, 1),
        "actb": np.tile(np.array(
            [1e-30, 1e-6, -707.10678] + [-50.0 * m * m for m in MUS],
            dtype=np.float32), (128, 1)),
    }


def kernel(**inputs):
    from concourse import bass_utils

    if "nc" not in _CACHE:
        _CACHE["nc"] = _build_program()
    nc = _CACHE["nc"]

    args = {k: np.asarray(v) for k, v in inputs.items()}
    in_maps = [_prep_core(c, **args) for c in range(NCORES)]
    res = bass_utils.run_bass_kernel_spmd(nc, in_maps,
                                          core_ids=list(range(NCORES)))
    out = np.concatenate([res.results[c]["out"].reshape(BPC)
                          for c in range(NCORES)])
    return out.reshape(B, 1).astype(np.float32)

